# revision 7
# baseline (speedup 1.0000x reference)
"""AnomalyAttention Trainium2 kernel (8 NeuronCores, SPMD data-parallel over batch).

Math (per b,h):
  series = softmax(causal_mask(Q K^T / 8))          = E / sum(E)
  prior  = rownorm(exp(-(l-s)^2 / (2 sigma'^2)))    (banded: |l-s|<=16 exact)
  out    = a*(E@V) + b*(G@V),  a = g/sum(E), b = (1-g)/sum(G)  per row.

Structure (per core: 2 batches x 8 heads; ACT/exp is the bottleneck):
  - sigma DMA issued first on the SP HWDGE queue; m-chain shortened:
    p3 = exp(LN3*sg + LN3*1e-5) (bias-folds the +1e-5), -0.5 folded
    into the d2 constant so the premul uses r2 = 1/sigp^2 directly.
  - Q/K head-0 block one-hop: SWDGE cast DMA HBM->SBUF natural bf16
    (kn0/qn0), 4 PE transposes per tensor into a 256-f32 "carve" of
    the first two score PSUM tiles (chunk c3 wraps to cols [1280:1344)
    which the sums cols [1280:1288) later overwrite, WAR-ordered),
    then 2 DVE copies -> KT/QT [128,512] bf16.  Remaining blocks go
    bf16-DRAM-scratch + xbar-transpose (cheap engine-side); bi1's
    scratch casts split [0:128)/[128:512) so its th0 lands early.
  - scores TRANSPOSED (S^T = K Q^T) packed into ONE 3-bank PSUM tile
    [128,1536] (col order sj0|sj1|sj3|sj2) -> ONE cross-bank exp per
    head (1252ns) -> ET bf16 lhsT.  Mask matmuls are emitted BEFORE
    the data matmuls in each accumulation group; heads 0-1 instead
    zero the exp'd diagonal junk on idle DVE (cold PE clock).
  - Gaussian prior via the alignment-safe "pair" scheme: premul
    (-0.5 d^2)*r2 in [l,s] pair layout, bf16 PE transposes for BOTH
    heads of a pair into one PSUM bank (bitcast view), ONE exp
    [128,1024] per pair -> GT2 bf16; U2 = 8 aligned 128-contraction
    matmuls per head.  Pair-0 premuls+exp hoisted ahead of the Tt
    loads so the ACT lead-in stays busy.
  - V: Vn split c0 / c1-3; Tt overlapping windows loaded as TWO
    stride-64 half-grid DMAs + 2 edge partials per batch (4 triggers
    instead of 8); edge memsets on DVE for bi0, Pool for bi1 (keeps
    them off the lead-in DVE critical path).
  - row sums via ones-column matmuls into S cols [1280:1288); one DVE
    reciprocal yields both 1/sums.  Gates broadcast through the same
    cols at (bi0,h0).
  - normalization: DVE copies U (PSUM->SBUF, GPSIMD cannot read
    PSUM); Pool computes t2 for all chunks, then chunks 0-1 combine
    on DVE (scalar_tensor_tensor) and chunks 2-3 on Pool (t1+add).
  - output stored bf16 (halves DMA traffic), cast back to f32 on
    host.  Heads 0-3 DMA at h==3, heads 4-7 per chunk after h==7,
    fanned across SP/ACT/Pool queues for the last batch.  The last
    head's U2 chunks 1-3 go to its own S-tile carve and copy out
    during the exp; per-chunk U1 copies + combines shorten the tail.
"""

import math
from contextlib import ExitStack

import ml_dtypes
import numpy as np

import concourse.bass as bass
import concourse.mybir as mybir
import concourse.tile as tile
from concourse import bacc
from concourse.bass_utils import run_bass_kernel_spmd

F32 = mybir.dt.float32
BF16 = mybir.dt.bfloat16
AF = mybir.ActivationFunctionType
OP = mybir.AluOpType

B, L, H, E = 16, 512, 8, 64
NCORES = 8
BPC = B // NCORES  # batches per core
PC = 128
NCH = 4
POISON = 1e12
LN3 = math.log(3.0)

# packed score-tile layout: sj0 [0:512], sj1 [512:896], sj3 [896:1024],
# sj2 [1024:1280]
SJ_BASE = {0: 0, 1: 512, 3: 896, 2: 1024}
# staging carve (f32 cols) for the th0 PE-transpose chunks: c3 wraps low
CARVE = {0: 1344, 1: 1408, 2: 1472, 3: 1280}

_CACHE = {}
LAST_RESULT = None


def _et_col(sj, li):
    """Packed ET column of l = 128*li for s-block sj (li >= sj)."""
    return SJ_BASE[sj] + 128 * (li - sj)


def _consts():
    ident = np.eye(PC, dtype=ml_dtypes.bfloat16)
    # mask for S^T diag block: -240 where l < s (strict lower triangle)
    mtri = np.tril(np.full((PC, PC), -240.0, dtype=np.float32), k=-1).astype(
        ml_dtypes.bfloat16
    )
    # d2pm[p, 128k + j] = -0.5*((p%64)+32-j)^2, poisoned outside |d|<=16
    # and seq edges; premul by r2 = 1/sigp^2 gives m*d^2 directly.
    p = np.arange(PC)[:, None]
    j = np.arange(PC)[None, :]
    d = (p % 64) + 32 - j
    base = np.where(np.abs(d) <= 16, (-0.5 * d * d).astype(np.float32), -POISON)
    d2pm = np.zeros((PC, 4 * PC), np.float32)
    for k in range(4):
        blk = base.copy()
        q2 = 2 * k + p // 64  # pair index per partition
        s = 64 * q2 - 32 + j
        blk[(s < 0) | (s >= L)] = -POISON
        d2pm[:, PC * k:PC * (k + 1)] = blk
    ones_col = np.ones((PC, 1), dtype=ml_dtypes.bfloat16)
    tri01 = np.triu(np.ones((PC, PC), dtype=np.float32)).astype(ml_dtypes.bfloat16)
    return ident, mtri, d2pm, ones_col, tri01


def _build():
    if "nc" in _CACHE:
        return _CACHE["nc"]
    nc = bacc.Bacc()
    ident_np, mtri_np, d2pm_np, ones_np, tri01_np = _consts()

    q_h = nc.dram_tensor("queries", [BPC, L, H, E], F32, kind="ExternalInput")
    k_h = nc.dram_tensor("keys", [BPC, L, H, E], F32, kind="ExternalInput")
    v_h = nc.dram_tensor("values", [BPC, L, H, E], F32, kind="ExternalInput")
    sig_h = nc.dram_tensor("sigma", [BPC, L, H], F32, kind="ExternalInput")
    hgl_h = nc.dram_tensor("hgl", [1, H], F32, kind="ExternalInput")
    out_h = nc.dram_tensor("out", [BPC, L, H, E], BF16, kind="ExternalOutput")

    ident_d = nc.inline_tensor(ident_np, name="identc")
    mtri_d = nc.inline_tensor(mtri_np, name="mtric")
    d2pm_d = nc.inline_tensor(d2pm_np, name="d2pmc")
    ones_d = nc.inline_tensor(ones_np, name="onesc")
    tri01_d = nc.inline_tensor(tri01_np, name="tri01c")

    with ExitStack() as ctx:
        tc = ctx.enter_context(tile.TileContext(nc))
        const = ctx.enter_context(tc.tile_pool(name="const", bufs=1))
        qkT = ctx.enter_context(tc.tile_pool(name="qkT", bufs=2))
        knp = ctx.enter_context(tc.tile_pool(name="knp", bufs=1))
        vpool = ctx.enter_context(tc.tile_pool(name="vpool", bufs=3))
        spool = ctx.enter_context(tc.tile_pool(name="spool", bufs=1))
        etpool = ctx.enter_context(tc.tile_pool(name="etpool", bufs=3))
        pmpool = ctx.enter_context(tc.tile_pool(name="pmpool", bufs=5))
        gtpool = ctx.enter_context(tc.tile_pool(name="gtpool", bufs=4))
        ucpool = ctx.enter_context(tc.tile_pool(name="ucpool", bufs=4))
        small = ctx.enter_context(tc.tile_pool(name="small", bufs=12))
        outp = ctx.enter_context(tc.tile_pool(name="outp", bufs=2))
        ps_s = ctx.enter_context(tc.tile_pool(name="ps_s", bufs=2, space="PSUM"))
        ps_u = ctx.enter_context(tc.tile_pool(name="ps_u", bufs=1, space="PSUM"))
        ps_g = ctx.enter_context(tc.tile_pool(name="ps_g", bufs=1, space="PSUM"))
        dram = ctx.enter_context(tc.tile_pool(name="dram", bufs=2, space="DRAM"))

        # ---- sigma DMA first on SP: it binds the ACT-side m chain ----
        SW = BPC * NCH * H  # 64: col = b*32 + c*8 + h ; partition p -> l=128c+p
        sraw = spool.tile([PC, SW], F32, tag="sraw")
        nc.sync.dma_start(
            sraw[:, :].rearrange("p (b c h) -> p b c h", b=BPC, c=NCH),
            sig_h[:, :, :].rearrange("b (c p) h -> p b c h", p=PC),
        )

        # ---- first q/k th-block casts (head-0/1 critical path) ----
        kn0 = knp.tile([PC, NCH, PC], BF16, tag="kn0")
        qn0 = knp.tile([PC, NCH, PC], BF16, tag="qn0")
        nc.gpsimd.dma_start(
            kn0, k_h[0, :, 0:2, :].rearrange("(c p) h e -> p c (h e)", p=PC)
        )
        nc.gpsimd.dma_start(
            qn0, q_h[0, :, 0:2, :].rearrange("(c p) h e -> p c (h e)", p=PC)
        )

        # ---- dummy exp at t~0: preloads the ACT Exp table ----
        dm = const.tile([1, 8], F32, tag="dm")
        nc.vector.memset(dm, 0.0)
        dme = const.tile([1, 8], F32, tag="dme")
        nc.scalar.activation(dme, dm, AF.Exp)

        # ---- constants on SP behind sigma ----
        hgl_sb = const.tile([1, H], F32, tag="hgl")
        nc.sync.dma_start(hgl_sb, hgl_h[:, :])
        ident = const.tile([PC, PC], BF16, tag="ident")
        nc.sync.dma_start(ident, ident_d[:, :])
        d2pm = const.tile([PC, 4 * PC], F32, tag="d2pm")
        nc.sync.dma_start(d2pm, d2pm_d[:, :])
        mtri = const.tile([PC, PC], BF16, tag="mtri")
        nc.sync.dma_start(mtri, mtri_d[:, :])
        tri01 = const.tile([PC, PC], BF16, tag="tri01")
        nc.sync.dma_start(tri01, tri01_d[:, :])
        ones_col = const.tile([PC, 1], BF16, tag="ones")
        nc.sync.dma_start(ones_col, ones_d[:, :])

        # ---- m chain: r2 = 1/sigp^2 (the -0.5 lives in d2pm) ----
        e5 = spool.tile([PC, SW], F32, tag="e5")
        nc.scalar.activation(e5, sraw, AF.Exp, scale=-5.0)
        p1 = spool.tile([PC, SW], F32, tag="p1")
        nc.vector.tensor_scalar_add(p1, e5, 1.0)
        sg = spool.tile([PC, SW], F32, tag="sg")
        nc.vector.reciprocal(sg, p1)
        b3 = spool.tile([PC, 1], F32, tag="b3")
        nc.vector.memset(b3, LN3 * 1e-5)
        p3 = spool.tile([PC, SW], F32, tag="p3")
        nc.scalar.activation(p3, sg, AF.Exp, scale=LN3, bias=b3[:, :])
        sm1 = spool.tile([PC, SW], F32, tag="sm1")
        nc.vector.tensor_scalar_add(sm1, p3, -1.0)
        s2 = spool.tile([PC, SW], F32, tag="s2")
        nc.vector.tensor_tensor(s2, sm1, sm1, OP.mult)
        r2 = spool.tile([PC, SW], F32, tag="r2")
        nc.vector.reciprocal(r2, s2)

        # ---- gate scalars (broadcast happens at bi0/h0 via S cols) ----
        ge = const.tile([1, H], F32, tag="ge")
        nc.scalar.activation(ge, hgl_sb, AF.Exp, scale=-1.0)
        gp = const.tile([1, H], F32, tag="gp")
        nc.vector.tensor_scalar_add(gp, ge, 1.0)
        gate = const.tile([1, H], F32, tag="gate")
        nc.vector.reciprocal(gate, gp)
        gates_b = const.tile([PC, H], F32, tag="gatesb")
        omg_b = const.tile([PC, H], F32, tag="omgb")
        onesf = const.tile([1, PC], F32, tag="onesf")
        nc.vector.memset(onesf, 1.0)

        for bi in range(BPC):
            # ---- Q/K blocks beyond th0/bi0: bf16 DRAM scratch + xbar ----
            qscr = dram.tile([L, H * E], BF16, tag="qscr")
            kscr = dram.tile([L, H * E], BF16, tag="kscr")
            if bi == 0:
                nc.gpsimd.dma_start(qscr[:, 128:512], q_h[bi, :, 2:8, :])
                nc.gpsimd.dma_start(kscr[:, 128:512], k_h[bi, :, 2:8, :])
            else:
                nc.gpsimd.dma_start(kscr[:, 0:128], k_h[bi, :, 0:2, :])
                nc.gpsimd.dma_start(qscr[:, 0:128], q_h[bi, :, 0:2, :])
                nc.gpsimd.dma_start(kscr[:, 128:512], k_h[bi, :, 2:8, :])
                nc.gpsimd.dma_start(qscr[:, 128:512], q_h[bi, :, 2:8, :])

            # ---- V natural chunks: c0 split early ----
            Vn = vpool.tile([PC, NCH, H * E], BF16, tag="vn")
            _v_src = v_h[bi, :, :, :].rearrange("(c p) h e -> p c (h e)", p=PC)
            nc.gpsimd.dma_start(Vn[:, 0:1, :], _v_src[:, 0:1, :])
            nc.gpsimd.dma_start(Vn[:, 1:NCH, :], _v_src[:, 1:NCH, :])

            # ---- pair premuls for pairs 0-1 + pair-0 hoist ----
            mo = bi * 32
            hoistG = {}
            pms = {}
            for hh in range(4):
                PMh = pmpool.tile([PC, 512], BF16, tag="PM")
                for k in range(NCH):
                    nc.gpsimd.tensor_scalar_mul(
                        PMh[:, PC * k:PC * (k + 1)],
                        d2pm[:, PC * k:PC * (k + 1)],
                        r2[:, mo + k * H + hh:mo + k * H + hh + 1],
                    )
                pms[hh] = PMh
            GS0 = ps_g.tile([PC, 512], F32, tag="GS")
            GS0b = GS0[:, 0:512].bitcast(BF16)
            for hh in (0, 1):
                go = 512 * hh
                for k in range(NCH):
                    nc.tensor.transpose(
                        GS0b[:, go + PC * k:go + PC * (k + 1)],
                        pms[hh][:, PC * k:PC * (k + 1)], ident,
                    )
            GT20 = gtpool.tile([PC, 1024], BF16, tag="GTs")
            nc.scalar.activation(GT20, GS0b[:, :], AF.Exp)
            hoistG = {0: GT20, 2: (pms[2], pms[3])}

            # ---- Tt pair family: two stride-64 half-grid DMAs + edges ----
            Tt = vpool.tile([PC, 8, H * E], BF16, tag="tt")
            _t_rows = v_h[bi, :, :, :].rearrange("r h e -> r (h e)")
            nc.gpsimd.dma_start(
                Tt[0:64, 1:7, :],
                _t_rows[32:416, :].rearrange("(j p) c -> p j c", p=64),
            )
            nc.gpsimd.dma_start(
                Tt[64:PC, 1:7, :],
                _t_rows[96:480, :].rearrange("(j p) c -> p j c", p=64),
            )
            nc.gpsimd.dma_start(Tt[32:PC, 0:1, :], _t_rows[0:96, :])
            nc.gpsimd.dma_start(Tt[0:96, 7:8, :], _t_rows[416:512, :])
            if bi == 0:
                nc.vector.memset(Tt[0:32, 0:1, :], 0.0)
                nc.vector.memset(Tt[96:PC, 7:8, :], 0.0)
            else:
                nc.gpsimd.memset(Tt[0:32, 0:1, :], 0.0)
                nc.gpsimd.memset(Tt[96:PC, 7:8, :], 0.0)

            oslab = []
            for li in range(NCH):
                ot = outp.tile([PC, H * E], BF16, tag=f"o{li}")
                oslab.append(ot)

            for hp in range(4):
                Se = ps_s.tile([PC, 1536], F32, tag="S")
                So = ps_s.tile([PC, 1536], F32, tag="S")
                if bi == 0 and hp == 0:
                    # one-hop th0: PE transposes into the S carves
                    for c in range(NCH):
                        nc.tensor.transpose(
                            Se[:, CARVE[c]:CARVE[c] + 64].bitcast(BF16),
                            kn0[:, c, :], ident,
                        )
                    KT = qkT.tile([PC, L], BF16, tag="kT")
                    nc.vector.tensor_copy(
                        KT[:, 0:384], Se[:, 1344:1536].bitcast(BF16)
                    )
                    nc.vector.tensor_copy(
                        KT[:, 384:512], Se[:, 1280:1344].bitcast(BF16)
                    )
                    for c in range(NCH):
                        nc.tensor.transpose(
                            So[:, CARVE[c]:CARVE[c] + 64].bitcast(BF16),
                            qn0[:, c, :], ident,
                        )
                    QT = qkT.tile([PC, L], BF16, tag="qT")
                    nc.vector.tensor_copy(
                        QT[:, 0:384], So[:, 1344:1536].bitcast(BF16)
                    )
                    nc.vector.tensor_copy(
                        QT[:, 384:512], So[:, 1280:1344].bitcast(BF16)
                    )
                else:
                    KT = qkT.tile([PC, L], BF16, tag="kT")
                    QT = qkT.tile([PC, L], BF16, tag="qT")
                    nc.sync.dma_start_transpose(
                        KT, kscr[:, hp * PC:(hp + 1) * PC]
                    )
                    nc.sync.dma_start_transpose(
                        QT, qscr[:, hp * PC:(hp + 1) * PC]
                    )

                for h in (2 * hp, 2 * hp + 1):
                    S = Se if h % 2 == 0 else So
                    po = 64 * (h % 2)

                    # ---- G path (pair-merged) ----
                    if h % 2 == 0:
                        if h == 0:
                            GT2 = hoistG[0]
                        else:
                            GS = ps_g.tile([PC, 512], F32, tag="GS")
                            GSb = GS[:, 0:512].bitcast(BF16)
                            for hh in (h, h + 1):
                                if h == 2:
                                    PM = hoistG[2][hh - 2]
                                else:
                                    PM = pmpool.tile([PC, 512], BF16, tag="PM")
                                    for k in range(NCH):
                                        nc.gpsimd.tensor_scalar_mul(
                                            PM[:, PC * k:PC * (k + 1)],
                                            d2pm[:, PC * k:PC * (k + 1)],
                                            r2[:, mo + k * H + hh:
                                               mo + k * H + hh + 1],
                                        )
                                go = 512 * (hh % 2)
                                for k in range(NCH):
                                    nc.tensor.transpose(
                                        GSb[:, go + PC * k:go + PC * (k + 1)],
                                        PM[:, PC * k:PC * (k + 1)], ident,
                                    )
                            GT2 = gtpool.tile([PC, 1024], BF16, tag="GTs")
                            nc.scalar.activation(GT2, GSb[:, :], AF.Exp)

                    # ---- packed scores + ONE exp ----
                    first = bi == 0 and h <= 1
                    last = bi == BPC - 1 and h == H - 1
                    goff = 512 * (h % 2)
                    U = ps_u.tile([PC, 512], F32, tag="U")
                    Uc = ucpool.tile([PC, 512], F32, tag="Uc")
                    if last:
                        # U2 chunks 1-3 prefill this head's own S carve and
                        # copy out during the exp; only chunk 0 (and U1)
                        # trails the final exp.
                        for q in range(8):
                            k = q // 2
                            pb = 64 * (q & 1)
                            dst = (U[pb:pb + 64, 256:320] if k == 0 else
                                   S[pb:pb + 64, 1224 + 64 * k:1288 + 64 * k])
                            nc.tensor.matmul(
                                dst,
                                GT2[:, goff + 64 * q:goff + 64 * q + 64],
                                Tt[:, q:q + 1, 64 * h:64 * h + 64],
                                start=True, stop=True,
                            )
                        nc.vector.tensor_copy(
                            Uc[:, 320:512], S[:, 1288:1480]
                        )
                    for sj in range(4):
                        cb = SJ_BASE[sj]
                        lh = KT[po:po + 64, sj * PC:(sj + 1) * PC]
                        if first:
                            nc.tensor.matmul(
                                S[:, cb:cb + PC], lh,
                                QT[po:po + 64, sj * PC:(sj + 1) * PC],
                                start=True, stop=True,
                            )
                        else:
                            nc.tensor.matmul(
                                S[:, cb:cb + PC], ident, mtri,
                                start=True, stop=False,
                            )
                            nc.tensor.matmul(
                                S[:, cb:cb + PC], lh,
                                QT[po:po + 64, sj * PC:(sj + 1) * PC],
                                start=False, stop=True,
                            )
                        nw = L - (sj + 1) * PC
                        if nw > 0:
                            nc.tensor.matmul(
                                S[:, cb + PC:cb + PC + nw], lh,
                                QT[po:po + 64, (sj + 1) * PC:L],
                                start=True, stop=True,
                            )
                    ET = etpool.tile([PC, 1280], BF16, tag="ET")
                    nc.scalar.activation(ET, S[:, 0:1280], AF.Exp, scale=0.125)
                    if first:
                        # heads 0-1: causal mask applied post-exp on idle DVE
                        for sj in range(4):
                            cb = SJ_BASE[sj]
                            nc.vector.tensor_tensor(
                                ET[:, cb:cb + PC], ET[:, cb:cb + PC], tri01,
                                OP.mult,
                            )

                    # ---- row sums into S cols [1280:1288) ----
                    sums = S[:, 1280:1288]
                    if bi == 0 and h == 0:
                        nc.tensor.matmul(sums[:, 0:H], onesf, gate,
                                         start=True, stop=True)
                        nc.vector.tensor_copy(gates_b, sums[:, 0:H])
                        nc.vector.tensor_scalar(
                            omg_b, gates_b, -1.0, 1.0, OP.mult, OP.add
                        )
                    for li in range(NCH):
                        for sj in range(li + 1):
                            ec = _et_col(sj, li)
                            nc.tensor.matmul(
                                sums[:, 4 + li:5 + li],
                                ET[:, ec:ec + PC], ones_col,
                                start=(sj == 0), stop=(sj == li),
                            )
                    for q in range(8):
                        k = q // 2
                        pb = 64 * (q & 1)
                        nc.tensor.matmul(
                            sums[pb:pb + 64, k:k + 1],
                            GT2[:, goff + 64 * q:goff + 64 * q + 64],
                            ones_col, start=True, stop=True,
                        )

                    # ---- U1 / U2 PV matmuls ----
                    for li in range(NCH):
                        for sj in range(li + 1):
                            ec = _et_col(sj, li)
                            nc.tensor.matmul(
                                U[:, 64 * li:64 * li + 64],
                                ET[:, ec:ec + PC],
                                Vn[:, sj:sj + 1, 64 * h:64 * h + 64],
                                start=(sj == 0), stop=(sj == li),
                            )
                    if not last:
                        for q in range(8):
                            k = q // 2
                            pb = 64 * (q & 1)
                            nc.tensor.matmul(
                                U[pb:pb + 64, 256 + 64 * k:256 + 64 * k + 64],
                                GT2[:, goff + 64 * q:goff + 64 * q + 64],
                                Tt[:, q:q + 1, 64 * h:64 * h + 64],
                                start=True, stop=True,
                            )

                    # ---- normalization: DVE copies PSUM out, Pool t2,
                    #      chunks 0-1 combine on DVE, 2-3 on Pool ----
                    rr = small.tile([PC, 8], F32, tag="rr")
                    nc.vector.reciprocal(rr, sums[:, :])
                    av = small.tile([PC, NCH], F32, tag="av")
                    nc.gpsimd.tensor_scalar_mul(av, rr[:, 4:8],
                                                gates_b[:, h:h + 1])
                    bv = small.tile([PC, NCH], F32, tag="bv")
                    nc.gpsimd.tensor_scalar_mul(bv, rr[:, 0:4],
                                                omg_b[:, h:h + 1])
                    if last:
                        nc.vector.tensor_copy(Uc[:, 256:320], U[:, 256:320])
                        for li in range(NCH):
                            nc.vector.tensor_copy(
                                Uc[:, 64 * li:64 * li + 64],
                                U[:, 64 * li:64 * li + 64],
                            )
                    else:
                        nc.vector.tensor_copy(Uc, U[:, :])
                    for li in range(NCH):
                        t2 = small.tile([PC, 64], F32, tag="t2")
                        nc.gpsimd.tensor_scalar_mul(
                            t2, Uc[:, 256 + 64 * li:256 + 64 * li + 64],
                            bv[:, li:li + 1],
                        )
                        if li < 2 and not last:
                            nc.vector.scalar_tensor_tensor(
                                oslab[li][:, 64 * h:64 * h + 64],
                                Uc[:, 64 * li:64 * li + 64],
                                av[:, li:li + 1], t2, OP.mult, OP.add,
                            )
                        else:
                            t1 = small.tile([PC, 64], F32, tag="t1")
                            nc.gpsimd.tensor_scalar_mul(
                                t1, Uc[:, 64 * li:64 * li + 64],
                                av[:, li:li + 1],
                            )
                            nc.gpsimd.tensor_tensor(
                                oslab[li][:, 64 * h:64 * h + 64], t1, t2,
                                OP.add,
                            )
                        if h == H - 1:
                            if bi == BPC - 1:
                                eng = (nc.sync, nc.scalar, nc.scalar,
                                       nc.gpsimd)[li]
                            else:
                                eng = nc.sync
                            eng.dma_start(
                                out_h[bi, li * PC:(li + 1) * PC, 4:8, :],
                                oslab[li][:, 256:512],
                            )

                    if h == 3:
                        for li in range(NCH):
                            nc.sync.dma_start(
                                out_h[bi, li * PC:(li + 1) * PC, 0:4, :],
                                oslab[li][:, 0:256],
                            )

    nc.compile()
    _CACHE["nc"] = nc
    return nc


def kernel(**inputs):
    global LAST_RESULT
    nc = _build()
    q = np.ascontiguousarray(inputs["queries"], dtype=np.float32)
    k = np.ascontiguousarray(inputs["keys"], dtype=np.float32)
    v = np.ascontiguousarray(inputs["values"], dtype=np.float32)
    sg = np.ascontiguousarray(inputs["sigma"], dtype=np.float32)
    hgl = np.ascontiguousarray(
        inputs["head_gate_logit"], dtype=np.float32
    ).reshape(1, H)

    in_maps = []
    for c in range(NCORES):
        b0 = BPC * c
        in_maps.append({
            "queries": q[b0:b0 + BPC],
            "keys": k[b0:b0 + BPC],
            "values": v[b0:b0 + BPC],
            "sigma": sg[b0:b0 + BPC],
            "hgl": hgl,
        })
    res = run_bass_kernel_spmd(nc, in_maps, core_ids=list(range(NCORES)))
    LAST_RESULT = res
    out = np.concatenate([r["out"] for r in res.results], axis=0)
    return np.asarray(out, dtype=np.float32)


# revision 10
# speedup vs baseline: 1.0022x; 1.0022x over previous
"""AnomalyAttention Trainium2 kernel (8 NeuronCores, SPMD data-parallel over batch).

Math (per b,h):
  series = softmax(causal_mask(Q K^T / 8))          = E / sum(E)
  prior  = rownorm(exp(-(l-s)^2 / (2 sigma'^2)))    (banded: |l-s|<=16 exact)
  out    = a*(E@V) + b*(G@V),  a = g/sum(E), b = (1-g)/sum(G)  per row.

Structure (per core: 2 batches x 8 heads; ACT/exp is the bottleneck):
  - sigma DMA issued first on the SP HWDGE queue; m-chain shortened:
    p3 = exp(LN3*sg + LN3*1e-5) (bias-folds the +1e-5), -0.5 folded
    into the d2 constant so the premul uses r2 = 1/sigp^2 directly.
  - Q/K head-0 block one-hop: SWDGE cast DMA HBM->SBUF natural bf16
    (kn0/qn0), 4 PE transposes per tensor into a 256-f32 "carve" of
    the first two score PSUM tiles (chunk c3 wraps to cols [1280:1344)
    which the sums cols [1280:1288) later overwrite, WAR-ordered),
    then 2 DVE copies -> KT/QT [128,512] bf16.  Remaining blocks go
    bf16-DRAM-scratch + xbar-transpose (cheap engine-side); bi1's
    scratch casts split [0:128)/[128:512) so its th0 lands early.
  - scores TRANSPOSED (S^T = K Q^T) packed into ONE 3-bank PSUM tile
    [128,1536] (col order sj0|sj1|sj3|sj2) -> ONE cross-bank exp per
    head (1252ns) -> ET bf16 lhsT.  Mask matmuls are emitted BEFORE
    the data matmuls in each accumulation group; heads 0-1 instead
    zero the exp'd diagonal junk on idle DVE (cold PE clock).
  - Gaussian prior via the alignment-safe "pair" scheme: premul
    (-0.5 d^2)*r2 in [l,s] pair layout, bf16 PE transposes for BOTH
    heads of a pair into one PSUM bank (bitcast view), ONE exp
    [128,1024] per pair -> GT2 bf16; U2 = 8 aligned 128-contraction
    matmuls per head.  Pair-0 premuls+exp hoisted ahead of the Tt
    loads so the ACT lead-in stays busy.
  - V: Vn split c0 / c1-3; Tt overlapping windows loaded as TWO
    stride-64 half-grid DMAs + 2 edge partials per batch (4 triggers
    instead of 8); edge memsets on DVE for bi0, Pool for bi1 (keeps
    them off the lead-in DVE critical path).
  - row sums via ones-column matmuls into S cols [1280:1288); one DVE
    reciprocal yields both 1/sums.  Gates broadcast through the same
    cols at (bi0,h0).
  - normalization: DVE copies U (PSUM->SBUF, GPSIMD cannot read
    PSUM); Pool computes t2 for all chunks, then chunks 0-1 combine
    on DVE (scalar_tensor_tensor) and chunks 2-3 on Pool (t1+add).
  - output stored bf16 (halves DMA traffic), cast back to f32 on
    host.  Heads 0-3 DMA at h==3, heads 4-7 per chunk after h==7,
    fanned across SP/ACT/Pool queues for the last batch.  The last
    head's U2 chunks 1-3 go to its own S-tile carve and copy out
    during the exp; per-chunk U1 copies + combines shorten the tail.
"""

import math
from contextlib import ExitStack

import ml_dtypes
import numpy as np

import concourse.bass as bass
import concourse.mybir as mybir
import concourse.tile as tile
from concourse import bacc
from concourse.bass_utils import run_bass_kernel_spmd

F32 = mybir.dt.float32
BF16 = mybir.dt.bfloat16
AF = mybir.ActivationFunctionType
OP = mybir.AluOpType

B, L, H, E = 16, 512, 8, 64
NCORES = 8
BPC = B // NCORES  # batches per core
PC = 128
NCH = 4
POISON = 1e12
LN3 = math.log(3.0)

# packed score-tile layout: sj0 [0:512], sj1 [512:896], sj3 [896:1024],
# sj2 [1024:1280]
SJ_BASE = {0: 0, 1: 512, 3: 896, 2: 1024}
# staging carve (f32 cols) for the th0 PE-transpose chunks: c3 wraps low
CARVE = {0: 1344, 1: 1408, 2: 1472, 3: 1280}

_CACHE = {}
LAST_RESULT = None


def _et_col(sj, li):
    """Packed ET column of l = 128*li for s-block sj (li >= sj)."""
    return SJ_BASE[sj] + 128 * (li - sj)


def _consts():
    ident = np.eye(PC, dtype=ml_dtypes.bfloat16)
    # mask for S^T diag block: -240 where l < s (strict lower triangle)
    mtri = np.tril(np.full((PC, PC), -240.0, dtype=np.float32), k=-1).astype(
        ml_dtypes.bfloat16
    )
    # d2pm[p, 128k + j] = -0.5*((p%64)+32-j)^2, poisoned outside |d|<=16
    # and seq edges; premul by r2 = 1/sigp^2 gives m*d^2 directly.
    p = np.arange(PC)[:, None]
    j = np.arange(PC)[None, :]
    d = (p % 64) + 32 - j
    base = np.where(np.abs(d) <= 16, (-0.5 * d * d).astype(np.float32), -POISON)
    d2pm = np.zeros((PC, 4 * PC), np.float32)
    for k in range(4):
        blk = base.copy()
        q2 = 2 * k + p // 64  # pair index per partition
        s = 64 * q2 - 32 + j
        blk[(s < 0) | (s >= L)] = -POISON
        d2pm[:, PC * k:PC * (k + 1)] = blk
    ones_col = np.ones((PC, 1), dtype=ml_dtypes.bfloat16)
    tri01 = np.triu(np.ones((PC, PC), dtype=np.float32)).astype(ml_dtypes.bfloat16)
    return ident, mtri, d2pm, ones_col, tri01


def _build():
    if "nc" in _CACHE:
        return _CACHE["nc"]
    nc = bacc.Bacc()
    ident_np, mtri_np, d2pm_np, ones_np, tri01_np = _consts()

    q_h = nc.dram_tensor("queries", [BPC, L, H, E], F32, kind="ExternalInput")
    k_h = nc.dram_tensor("keys", [BPC, L, H, E], F32, kind="ExternalInput")
    v_h = nc.dram_tensor("values", [BPC, L, H, E], F32, kind="ExternalInput")
    sig_h = nc.dram_tensor("sigma", [BPC, L, H], F32, kind="ExternalInput")
    hgl_h = nc.dram_tensor("hgl", [1, H], F32, kind="ExternalInput")
    out_h = nc.dram_tensor("out", [BPC, L, H, E], BF16, kind="ExternalOutput")

    ident_d = nc.inline_tensor(ident_np, name="identc")
    mtri_d = nc.inline_tensor(mtri_np, name="mtric")
    d2pm_d = nc.inline_tensor(d2pm_np, name="d2pmc")
    ones_d = nc.inline_tensor(ones_np, name="onesc")
    tri01_d = nc.inline_tensor(tri01_np, name="tri01c")

    with ExitStack() as ctx:
        tc = ctx.enter_context(tile.TileContext(nc))
        const = ctx.enter_context(tc.tile_pool(name="const", bufs=1))
        qkT = ctx.enter_context(tc.tile_pool(name="qkT", bufs=2))
        knp = ctx.enter_context(tc.tile_pool(name="knp", bufs=1))
        vpool = ctx.enter_context(tc.tile_pool(name="vpool", bufs=3))
        spool = ctx.enter_context(tc.tile_pool(name="spool", bufs=1))
        etpool = ctx.enter_context(tc.tile_pool(name="etpool", bufs=3))
        pmpool = ctx.enter_context(tc.tile_pool(name="pmpool", bufs=5))
        gtpool = ctx.enter_context(tc.tile_pool(name="gtpool", bufs=4))
        ucpool = ctx.enter_context(tc.tile_pool(name="ucpool", bufs=4))
        small = ctx.enter_context(tc.tile_pool(name="small", bufs=12))
        outp = ctx.enter_context(tc.tile_pool(name="outp", bufs=2))
        ps_s = ctx.enter_context(tc.tile_pool(name="ps_s", bufs=2, space="PSUM"))
        ps_u = ctx.enter_context(tc.tile_pool(name="ps_u", bufs=1, space="PSUM"))
        ps_g = ctx.enter_context(tc.tile_pool(name="ps_g", bufs=1, space="PSUM"))
        dram = ctx.enter_context(tc.tile_pool(name="dram", bufs=2, space="DRAM"))

        # ---- sigma DMA first on SP: it binds the ACT-side m chain ----
        SW = BPC * NCH * H  # 64: col = b*32 + c*8 + h ; partition p -> l=128c+p
        sraw = spool.tile([PC, SW], F32, tag="sraw")
        nc.sync.dma_start(
            sraw[:, :].rearrange("p (b c h) -> p b c h", b=BPC, c=NCH),
            sig_h[:, :, :].rearrange("b (c p) h -> p b c h", p=PC),
        )

        # ---- first q/k th-block casts (head-0/1 critical path) ----
        kn0 = knp.tile([PC, NCH, PC], BF16, tag="kn0")
        qn0 = knp.tile([PC, NCH, PC], BF16, tag="qn0")
        nc.gpsimd.dma_start(
            kn0, k_h[0, :, 0:2, :].rearrange("(c p) h e -> p c (h e)", p=PC)
        )
        nc.gpsimd.dma_start(
            qn0, q_h[0, :, 0:2, :].rearrange("(c p) h e -> p c (h e)", p=PC)
        )

        # ---- dummy exp at t~0: preloads the ACT Exp table ----
        dm = const.tile([1, 8], F32, tag="dm")
        nc.vector.memset(dm, 0.0)
        dme = const.tile([1, 8], F32, tag="dme")
        nc.scalar.activation(dme, dm, AF.Exp)

        # ---- constants on SP behind sigma ----
        hgl_sb = const.tile([1, H], F32, tag="hgl")
        nc.sync.dma_start(hgl_sb, hgl_h[:, :])
        ident = const.tile([PC, PC], BF16, tag="ident")
        nc.sync.dma_start(ident, ident_d[:, :])
        d2pm = const.tile([PC, 4 * PC], F32, tag="d2pm")
        nc.sync.dma_start(d2pm, d2pm_d[:, :])
        mtri = const.tile([PC, PC], BF16, tag="mtri")
        nc.sync.dma_start(mtri, mtri_d[:, :])
        tri01 = const.tile([PC, PC], BF16, tag="tri01")
        nc.sync.dma_start(tri01, tri01_d[:, :])
        ones_col = const.tile([PC, 1], BF16, tag="ones")
        nc.sync.dma_start(ones_col, ones_d[:, :])

        # ---- m chain: r2 = 1/sigp^2 (the -0.5 lives in d2pm) ----
        e5 = spool.tile([PC, SW], F32, tag="e5")
        nc.scalar.activation(e5, sraw, AF.Exp, scale=-5.0)
        p1 = spool.tile([PC, SW], F32, tag="p1")
        nc.vector.tensor_scalar_add(p1, e5, 1.0)
        sg = spool.tile([PC, SW], F32, tag="sg")
        nc.vector.reciprocal(sg, p1)
        b3 = spool.tile([PC, 1], F32, tag="b3")
        nc.vector.memset(b3, LN3 * 1e-5)
        p3 = spool.tile([PC, SW], F32, tag="p3")
        nc.scalar.activation(p3, sg, AF.Exp, scale=LN3, bias=b3[:, :])
        sm1 = spool.tile([PC, SW], F32, tag="sm1")
        nc.vector.tensor_scalar_add(sm1, p3, -1.0)
        s2 = spool.tile([PC, SW], F32, tag="s2")
        nc.vector.tensor_tensor(s2, sm1, sm1, OP.mult)
        r2 = spool.tile([PC, SW], F32, tag="r2")
        nc.vector.reciprocal(r2, s2)

        # ---- gate scalars (broadcast happens at bi0/h0 via S cols) ----
        ge = const.tile([1, H], F32, tag="ge")
        nc.scalar.activation(ge, hgl_sb, AF.Exp, scale=-1.0)
        gp = const.tile([1, H], F32, tag="gp")
        nc.vector.tensor_scalar_add(gp, ge, 1.0)
        gate = const.tile([1, H], F32, tag="gate")
        nc.vector.reciprocal(gate, gp)
        gates_b = const.tile([PC, H], F32, tag="gatesb")
        omg_b = const.tile([PC, H], F32, tag="omgb")
        onesf = const.tile([1, PC], F32, tag="onesf")
        nc.vector.memset(onesf, 1.0)

        for bi in range(BPC):
            mo = bi * 32

            def _premul_hoist():
                pms = {}
                for hh in range(4):
                    PMh = pmpool.tile([PC, 512], BF16, tag="PM")
                    for k in range(NCH):
                        nc.gpsimd.tensor_scalar_mul(
                            PMh[:, PC * k:PC * (k + 1)],
                            d2pm[:, PC * k:PC * (k + 1)],
                            r2[:, mo + k * H + hh:mo + k * H + hh + 1],
                        )
                    pms[hh] = PMh
                GS0 = ps_g.tile([PC, 512], F32, tag="GS")
                GS0b = GS0[:, 0:512].bitcast(BF16)
                for hh in (0, 1):
                    go = 512 * hh
                    for k in range(NCH):
                        nc.tensor.transpose(
                            GS0b[:, go + PC * k:go + PC * (k + 1)],
                            pms[hh][:, PC * k:PC * (k + 1)], ident,
                        )
                GT20 = gtpool.tile([PC, 1024], BF16, tag="GTs")
                nc.scalar.activation(GT20, GS0b[:, :], AF.Exp)
                return {0: GT20, 2: (pms[2], pms[3])}

            # bi1: premuls/GS hoist first so the Pool queue serves them
            # before its DMA triggers (r2 is long ready; closes the ACT
            # gap at the batch boundary)
            if bi > 0:
                hoistG = _premul_hoist()

            # ---- Q/K blocks beyond th0/bi0: bf16 DRAM scratch + xbar ----
            qscr = dram.tile([L, H * E], BF16, tag="qscr")
            kscr = dram.tile([L, H * E], BF16, tag="kscr")
            if bi == 0:
                nc.gpsimd.dma_start(qscr[:, 128:512], q_h[bi, :, 2:8, :])
                nc.gpsimd.dma_start(kscr[:, 128:512], k_h[bi, :, 2:8, :])
            else:
                nc.gpsimd.dma_start(kscr[:, 0:128], k_h[bi, :, 0:2, :])
                nc.gpsimd.dma_start(qscr[:, 0:128], q_h[bi, :, 0:2, :])
                nc.gpsimd.dma_start(kscr[:, 128:512], k_h[bi, :, 2:8, :])
                nc.gpsimd.dma_start(qscr[:, 128:512], q_h[bi, :, 2:8, :])

            # ---- V natural chunks: c0 split early ----
            Vn = vpool.tile([PC, NCH, H * E], BF16, tag="vn")
            _v_src = v_h[bi, :, :, :].rearrange("(c p) h e -> p c (h e)", p=PC)
            nc.gpsimd.dma_start(Vn[:, 0:1, :], _v_src[:, 0:1, :])
            nc.gpsimd.dma_start(Vn[:, 1:NCH, :], _v_src[:, 1:NCH, :])

            # bi0: premuls wait on r2 (~4us) so they go after the V trigger
            if bi == 0:
                hoistG = _premul_hoist()

            # ---- Tt pair family: two stride-64 half-grid DMAs + edges ----
            Tt = vpool.tile([PC, 8, H * E], BF16, tag="tt")
            _t_rows = v_h[bi, :, :, :].rearrange("r h e -> r (h e)")
            nc.gpsimd.dma_start(
                Tt[0:64, 1:7, :],
                _t_rows[32:416, :].rearrange("(j p) c -> p j c", p=64),
            )
            nc.gpsimd.dma_start(
                Tt[64:PC, 1:7, :],
                _t_rows[96:480, :].rearrange("(j p) c -> p j c", p=64),
            )
            nc.gpsimd.dma_start(Tt[32:PC, 0:1, :], _t_rows[0:96, :])
            nc.gpsimd.dma_start(Tt[0:96, 7:8, :], _t_rows[416:512, :])
            if bi == 0:
                nc.vector.memset(Tt[0:32, 0:1, :], 0.0)
                nc.vector.memset(Tt[96:PC, 7:8, :], 0.0)
            else:
                nc.gpsimd.memset(Tt[0:32, 0:1, :], 0.0)
                nc.gpsimd.memset(Tt[96:PC, 7:8, :], 0.0)

            oslab = []
            for li in range(NCH):
                ot = outp.tile([PC, H * E], BF16, tag=f"o{li}")
                oslab.append(ot)

            for hp in range(4):
                Se = ps_s.tile([PC, 1536], F32, tag="S")
                So = ps_s.tile([PC, 1536], F32, tag="S")
                if bi == 0 and hp == 0:
                    # one-hop th0: PE transposes into the S carves
                    for c in range(NCH):
                        nc.tensor.transpose(
                            Se[:, CARVE[c]:CARVE[c] + 64].bitcast(BF16),
                            kn0[:, c, :], ident,
                        )
                    KT = qkT.tile([PC, L], BF16, tag="kT")
                    nc.vector.tensor_copy(
                        KT[:, 0:384], Se[:, 1344:1536].bitcast(BF16)
                    )
                    nc.vector.tensor_copy(
                        KT[:, 384:512], Se[:, 1280:1344].bitcast(BF16)
                    )
                    for c in range(NCH):
                        nc.tensor.transpose(
                            So[:, CARVE[c]:CARVE[c] + 64].bitcast(BF16),
                            qn0[:, c, :], ident,
                        )
                    QT = qkT.tile([PC, L], BF16, tag="qT")
                    nc.vector.tensor_copy(
                        QT[:, 0:384], So[:, 1344:1536].bitcast(BF16)
                    )
                    nc.vector.tensor_copy(
                        QT[:, 384:512], So[:, 1280:1344].bitcast(BF16)
                    )
                else:
                    KT = qkT.tile([PC, L], BF16, tag="kT")
                    QT = qkT.tile([PC, L], BF16, tag="qT")
                    nc.sync.dma_start_transpose(
                        KT, kscr[:, hp * PC:(hp + 1) * PC]
                    )
                    nc.sync.dma_start_transpose(
                        QT, qscr[:, hp * PC:(hp + 1) * PC]
                    )

                for h in (2 * hp, 2 * hp + 1):
                    S = Se if h % 2 == 0 else So
                    po = 64 * (h % 2)

                    # ---- G path (pair-merged) ----
                    if h % 2 == 0:
                        if h == 0:
                            GT2 = hoistG[0]
                        else:
                            GS = ps_g.tile([PC, 512], F32, tag="GS")
                            GSb = GS[:, 0:512].bitcast(BF16)
                            for hh in (h, h + 1):
                                if h == 2:
                                    PM = hoistG[2][hh - 2]
                                else:
                                    PM = pmpool.tile([PC, 512], BF16, tag="PM")
                                    for k in range(NCH):
                                        nc.gpsimd.tensor_scalar_mul(
                                            PM[:, PC * k:PC * (k + 1)],
                                            d2pm[:, PC * k:PC * (k + 1)],
                                            r2[:, mo + k * H + hh:
                                               mo + k * H + hh + 1],
                                        )
                                go = 512 * (hh % 2)
                                for k in range(NCH):
                                    nc.tensor.transpose(
                                        GSb[:, go + PC * k:go + PC * (k + 1)],
                                        PM[:, PC * k:PC * (k + 1)], ident,
                                    )
                            GT2 = gtpool.tile([PC, 1024], BF16, tag="GTs")
                            nc.scalar.activation(GT2, GSb[:, :], AF.Exp)

                    # ---- packed scores + ONE exp ----
                    first = bi == 0 and h <= 1
                    last = bi == BPC - 1 and h == H - 1
                    goff = 512 * (h % 2)
                    U = ps_u.tile([PC, 512], F32, tag="U")
                    Uc = ucpool.tile([PC, 512], F32, tag="Uc")
                    if last:
                        # U2 prefills the hp3 S carves (chunk 0 -> Se, 1-3
                        # -> So) and copies out during the exp, so only U1
                        # trails the final exp.  Avoids ps_u serialization.
                        for q in range(8):
                            k = q // 2
                            pb = 64 * (q & 1)
                            dst = (Se[pb:pb + 64, 1288:1352] if k == 0 else
                                   S[pb:pb + 64, 1224 + 64 * k:1288 + 64 * k])
                            nc.tensor.matmul(
                                dst,
                                GT2[:, goff + 64 * q:goff + 64 * q + 64],
                                Tt[:, q:q + 1, 64 * h:64 * h + 64],
                                start=True, stop=True,
                            )
                        nc.vector.tensor_copy(Uc[:, 256:320], Se[:, 1288:1352])
                        nc.vector.tensor_copy(Uc[:, 320:512], S[:, 1288:1480])
                    for sj in range(4):
                        cb = SJ_BASE[sj]
                        lh = KT[po:po + 64, sj * PC:(sj + 1) * PC]
                        if first:
                            nc.tensor.matmul(
                                S[:, cb:cb + PC], lh,
                                QT[po:po + 64, sj * PC:(sj + 1) * PC],
                                start=True, stop=True,
                            )
                        else:
                            nc.tensor.matmul(
                                S[:, cb:cb + PC], ident, mtri,
                                start=True, stop=False,
                            )
                            nc.tensor.matmul(
                                S[:, cb:cb + PC], lh,
                                QT[po:po + 64, sj * PC:(sj + 1) * PC],
                                start=False, stop=True,
                            )
                        nw = L - (sj + 1) * PC
                        if nw > 0:
                            nc.tensor.matmul(
                                S[:, cb + PC:cb + PC + nw], lh,
                                QT[po:po + 64, (sj + 1) * PC:L],
                                start=True, stop=True,
                            )
                    ET = etpool.tile([PC, 1280], BF16, tag="ET")
                    nc.scalar.activation(ET, S[:, 0:1280], AF.Exp, scale=0.125)
                    if first:
                        # heads 0-1: causal mask applied post-exp on idle DVE
                        for sj in range(4):
                            cb = SJ_BASE[sj]
                            nc.vector.tensor_tensor(
                                ET[:, cb:cb + PC], ET[:, cb:cb + PC], tri01,
                                OP.mult,
                            )

                    # ---- row sums into S cols [1280:1288) ----
                    sums = S[:, 1280:1288]
                    if bi == 0 and h == 0:
                        nc.tensor.matmul(sums[:, 0:H], onesf, gate,
                                         start=True, stop=True)
                        nc.vector.tensor_copy(gates_b, sums[:, 0:H])
                        nc.vector.tensor_scalar(
                            omg_b, gates_b, -1.0, 1.0, OP.mult, OP.add
                        )
                    for li in range(NCH):
                        for sj in range(li + 1):
                            ec = _et_col(sj, li)
                            nc.tensor.matmul(
                                sums[:, 4 + li:5 + li],
                                ET[:, ec:ec + PC], ones_col,
                                start=(sj == 0), stop=(sj == li),
                            )
                    for q in range(8):
                        k = q // 2
                        pb = 64 * (q & 1)
                        nc.tensor.matmul(
                            sums[pb:pb + 64, k:k + 1],
                            GT2[:, goff + 64 * q:goff + 64 * q + 64],
                            ones_col, start=True, stop=True,
                        )

                    # ---- U1 / U2 PV matmuls ----
                    for li in range(NCH):
                        for sj in range(li + 1):
                            ec = _et_col(sj, li)
                            nc.tensor.matmul(
                                U[:, 64 * li:64 * li + 64],
                                ET[:, ec:ec + PC],
                                Vn[:, sj:sj + 1, 64 * h:64 * h + 64],
                                start=(sj == 0), stop=(sj == li),
                            )
                    if not last:
                        for q in range(8):
                            k = q // 2
                            pb = 64 * (q & 1)
                            nc.tensor.matmul(
                                U[pb:pb + 64, 256 + 64 * k:256 + 64 * k + 64],
                                GT2[:, goff + 64 * q:goff + 64 * q + 64],
                                Tt[:, q:q + 1, 64 * h:64 * h + 64],
                                start=True, stop=True,
                            )

                    # ---- normalization: DVE copies PSUM out, Pool t2,
                    #      chunks 0-1 combine on DVE, 2-3 on Pool ----
                    rr = small.tile([PC, 8], F32, tag="rr")
                    nc.vector.reciprocal(rr, sums[:, :])
                    av = small.tile([PC, NCH], F32, tag="av")
                    nc.gpsimd.tensor_scalar_mul(av, rr[:, 4:8],
                                                gates_b[:, h:h + 1])
                    bv = small.tile([PC, NCH], F32, tag="bv")
                    nc.gpsimd.tensor_scalar_mul(bv, rr[:, 0:4],
                                                omg_b[:, h:h + 1])
                    if last:
                        for li in range(NCH):
                            nc.vector.tensor_copy(
                                Uc[:, 64 * li:64 * li + 64],
                                U[:, 64 * li:64 * li + 64],
                            )
                    else:
                        nc.vector.tensor_copy(Uc, U[:, :])
                    for li in range(NCH):
                        t2 = small.tile([PC, 64], F32, tag="t2")
                        nc.gpsimd.tensor_scalar_mul(
                            t2, Uc[:, 256 + 64 * li:256 + 64 * li + 64],
                            bv[:, li:li + 1],
                        )
                        if li < 2 and not last:
                            nc.vector.scalar_tensor_tensor(
                                oslab[li][:, 64 * h:64 * h + 64],
                                Uc[:, 64 * li:64 * li + 64],
                                av[:, li:li + 1], t2, OP.mult, OP.add,
                            )
                        else:
                            t1 = small.tile([PC, 64], F32, tag="t1")
                            nc.gpsimd.tensor_scalar_mul(
                                t1, Uc[:, 64 * li:64 * li + 64],
                                av[:, li:li + 1],
                            )
                            nc.gpsimd.tensor_tensor(
                                oslab[li][:, 64 * h:64 * h + 64], t1, t2,
                                OP.add,
                            )
                        if h == H - 1:
                            if bi == BPC - 1:
                                eng = (nc.sync, nc.scalar, nc.scalar,
                                       nc.gpsimd)[li]
                            else:
                                eng = nc.sync
                            eng.dma_start(
                                out_h[bi, li * PC:(li + 1) * PC, 4:8, :],
                                oslab[li][:, 256:512],
                            )

                    if h == 3:
                        for li in range(NCH):
                            nc.sync.dma_start(
                                out_h[bi, li * PC:(li + 1) * PC, 0:4, :],
                                oslab[li][:, 0:256],
                            )

    nc.compile()
    _CACHE["nc"] = nc
    return nc


def kernel(**inputs):
    global LAST_RESULT
    nc = _build()
    q = np.ascontiguousarray(inputs["queries"], dtype=np.float32)
    k = np.ascontiguousarray(inputs["keys"], dtype=np.float32)
    v = np.ascontiguousarray(inputs["values"], dtype=np.float32)
    sg = np.ascontiguousarray(inputs["sigma"], dtype=np.float32)
    hgl = np.ascontiguousarray(
        inputs["head_gate_logit"], dtype=np.float32
    ).reshape(1, H)

    in_maps = []
    for c in range(NCORES):
        b0 = BPC * c
        in_maps.append({
            "queries": q[b0:b0 + BPC],
            "keys": k[b0:b0 + BPC],
            "values": v[b0:b0 + BPC],
            "sigma": sg[b0:b0 + BPC],
            "hgl": hgl,
        })
    res = run_bass_kernel_spmd(nc, in_maps, core_ids=list(range(NCORES)))
    LAST_RESULT = res
    out = np.concatenate([r["out"] for r in res.results], axis=0)
    return np.asarray(out, dtype=np.float32)
